# revision 17
# baseline (speedup 1.0000x reference)
"""Trainium2 Bass kernel for nn_Block_2010044694563 (dense transformer block).

B=4, S=2048, D=768, H=12 heads of 64. 8 NeuronCores, no collectives:
core c handles batch c//2, query-half c%2. Each core receives its batch's
2048 tokens rolled so its 1024 query rows come first, computes LN1 + K/V
over all 2048 local tokens (the only redundant work), attention for its
1024 queries x 12 heads, then out-proj + FFN on its 1024 rows.

Precision: fp32 storage / elementwise / PSUM accumulation; bf16 matmul
operands (full PE rate). LN gains/biases are folded into the weight
matrices on the host.

Layout: activations are kept feature-major ("transposed", h^T) via PE
transposes so that attention scores come out as scores^T [k, q]; the
attention mask and softmax exp are then fused into a single per-partition
ACT op, and PV with a ones-augmented V gives softmax denominators free.
"""

import numpy as np
import ml_dtypes

B, S, D, H = 4, 2048, 768, 12
HS = D // H           # 64
P = 128
NT = S                # local tokens per core (whole batch)
NQ = S // 2           # query tokens per core
TCH = NT // P         # 16 token chunks
QCH = NQ // P         # 8 query chunks
KC = D // P           # 6 feature chunks
EPS = 1e-5
NEG = -1e9
SCALE = float(D) ** -0.5
BF16 = ml_dtypes.bfloat16

_PROGRAM_CACHE = {}


def _build_program():
    import concourse.bass as bass
    import concourse.mybir as mybir
    import concourse.tile as tile
    from concourse import bacc
    from concourse.masks import make_identity
    from contextlib import ExitStack

    f32 = mybir.dt.float32
    bf16 = mybir.dt.bfloat16
    AF = mybir.ActivationFunctionType
    OP = mybir.AluOpType

    nc = bacc.Bacc(None, target_bir_lowering=False)

    x_d = nc.dram_tensor("x_local", [NT, D], f32, kind="ExternalInput")
    mb_d = nc.dram_tensor("maskbias", [NT], f32, kind="ExternalInput")
    wq_d = nc.dram_tensor("wq", [D, D], bf16, kind="ExternalInput")
    wk_d = nc.dram_tensor("wk", [D, D], bf16, kind="ExternalInput")
    wv_d = nc.dram_tensor("wv", [D, D], bf16, kind="ExternalInput")
    wo_d = nc.dram_tensor("wo", [D, D], bf16, kind="ExternalInput")
    w1_d = nc.dram_tensor("w1", [D, D], bf16, kind="ExternalInput")
    w2_d = nc.dram_tensor("w2", [D, D], bf16, kind="ExternalInput")
    bq_d = nc.dram_tensor("bq", [D], f32, kind="ExternalInput")
    bk_d = nc.dram_tensor("bk", [D], f32, kind="ExternalInput")
    bo_d = nc.dram_tensor("bo2", [D], f32, kind="ExternalInput")
    b1_d = nc.dram_tensor("b1f", [D], f32, kind="ExternalInput")
    b2_d = nc.dram_tensor("b2f", [D], f32, kind="ExternalInput")
    out_d = nc.dram_tensor("out", [NQ, D], f32, kind="ExternalOutput")

    with tile.TileContext(nc) as tc, ExitStack() as ctx:
        const = ctx.enter_context(tc.tile_pool(name="const", bufs=1))
        glob = ctx.enter_context(tc.tile_pool(name="glob", bufs=1))
        rot = ctx.enter_context(tc.tile_pool(name="rot", bufs=1))
        wpool = ctx.enter_context(tc.tile_pool(name="wpool", bufs=1))

        f16 = mybir.dt.float16

        # ---- constants ----
        ident = const.tile([P, P], bf16)
        make_identity(nc, ident)
        ones1 = const.tile([1, HS], f16)
        nc.vector.memset(ones1, 1.0)
        mb_sb = const.tile([P, TCH], f32)
        nc.sync.dma_start(out=mb_sb, in_=mb_d[:].rearrange("(c p) -> p c", p=P))
        bq_sb = const.tile([P, KC], f32)
        nc.sync.dma_start(out=bq_sb, in_=bq_d[:].rearrange("(c p) -> p c", p=P))
        bk_sb = const.tile([P, KC], f32)
        nc.sync.dma_start(out=bk_sb, in_=bk_d[:].rearrange("(c p) -> p c", p=P))
        b1_sb = const.tile([P, KC], f32)
        nc.sync.dma_start(out=b1_sb, in_=b1_d[:].rearrange("(c p) -> p c", p=P))
        # per-feature biases broadcast across partitions (token-major use)
        bo_b = const.tile([P, D], f32)
        _bo = bo_d[:]
        nc.gpsimd.dma_start(
            out=bo_b, in_=bass.AP(tensor=_bo.tensor, offset=_bo.offset, ap=[[0, P], _bo.ap[0]])
        )
        b2_b = const.tile([P, D], f32)
        _b2 = b2_d[:]
        nc.gpsimd.dma_start(
            out=b2_b, in_=bass.AP(tensor=_b2.tensor, offset=_b2.offset, ap=[[0, P], _b2.ap[0]])
        )

        # whole-kernel persistent: attention output (feature-major, normalized)
        oT = glob.tile([P, KC, NQ], bf16)
        xq = glob.tile([P, QCH, D], f32)

        x_r = x_d[:].rearrange("(c p) d -> c p d", p=P)
        for t in range(QCH):
            nc.sync.dma_start(out=xq[:, t], in_=x_r[t])
            nc.vector.tensor_tensor(xq[:, t], xq[:, t], bo_b, OP.add)

        # phase-scoped pools (stack order: apool outlives hpool)
        apool = tc.alloc_tile_pool(name="apool", bufs=1)
        hpool = tc.alloc_tile_pool(name="hpool", bufs=1)
        ps_a = tc.alloc_tile_pool(name="ps_a", bufs=1, space="PSUM")

        qT = apool.tile([P, KC, NQ], bf16)       # Q^T
        kT = apool.tile([P, KC, NT], bf16)       # K^T
        vA = apool.tile([P, TCH, H, HS + 1], bf16)  # V per (tok chunk, head): [V | 1]
        hT = hpool.tile([P, KC, NT], bf16)       # LN1(x)^T, feature-major

        nc.vector.memset(vA[:, :, :, HS : HS + 1], 1.0)

        # ================= Phase 1: LN1 + transpose to h^T =================
        with nc.named_scope("ln1"):
            for t in range(TCH):
                xt = rot.tile([P, D], f32, tag="xin", bufs=3, name=f"xt{t}")
                nc.sync.dma_start(out=xt, in_=x_r[t])
                scr = rot.tile([P, D], bf16, tag="xn", bufs=4, name=f"scr{t}")
                ssq = rot.tile([P, 1], f32, tag="ssq", bufs=4, name=f"ssq{t}")
                nc.scalar.activation(scr, xt, AF.Square, accum_out=ssq)
                msum = rot.tile([P, 1], f32, tag="msum", bufs=4, name=f"msum{t}")
                nc.vector.reduce_sum(out=msum, in_=xt, axis=mybir.AxisListType.X)
                # var = ssq/D - (msum/D)^2 ; rstd = sqrt(1/(var+eps))
                mu = rot.tile([P, 1], f32, tag="mu", bufs=4, name=f"mu{t}")
                nc.vector.tensor_scalar_mul(out=mu, in0=msum, scalar1=1.0 / D)
                mu2 = rot.tile([P, 1], f32, tag="mu2", bufs=4, name=f"mu2{t}")
                nc.vector.tensor_tensor(mu2, mu, mu, OP.mult)
                ve = rot.tile([P, 1], f32, tag="ve", bufs=4, name=f"ve_{t}")
                nc.vector.tensor_scalar(
                    out=ve, in0=ssq, scalar1=1.0 / D, scalar2=EPS,
                    op0=OP.mult, op1=OP.add,
                )
                nc.vector.tensor_tensor(ve, ve, mu2, OP.subtract)
                rstd = rot.tile([P, 1], f32, tag="rstd", bufs=4, name=f"rstd{t}")
                nc.vector.reciprocal_approx_fast(out=rstd, in_=ve)
                nc.scalar.activation(rstd, rstd, AF.Sqrt, scale=1.0)
                nmr = rot.tile([P, 1], f32, tag="nmr", bufs=4, name=f"nmr{t}")
                nc.vector.tensor_tensor(nmr, mu, rstd, OP.mult)
                nc.vector.tensor_scalar_mul(out=nmr, in0=nmr, scalar1=-1.0)
                xn = rot.tile([P, D], bf16, tag="xn", bufs=4, name=f"xn{t}")
                nc.scalar.activation(xn, xt, AF.Identity, bias=nmr, scale=rstd)
                pt = ps_a.tile([P, KC, P], bf16, tag="tp", bufs=3, name=f"pt{t}")
                for f in range(KC):
                    nc.tensor.transpose(pt[:, f], xn[:, f * P : (f + 1) * P], ident)
                nc.vector.tensor_copy(out=hT[:, :, t * P : (t + 1) * P], in_=pt)

        # ================= Phase 2: V projection =================
        with nc.named_scope("qkv"):
            wq_sb = wpool.tile([P, KC, D], bf16, tag="w", bufs=3, name="wq_sb")
            nc.sync.dma_start(out=wq_sb, in_=wq_d[:].rearrange("(c p) n -> p c n", p=P))
            wk_sb = wpool.tile([P, KC, D], bf16, tag="w", bufs=3, name="wk_sb")
            nc.sync.dma_start(out=wk_sb, in_=wk_d[:].rearrange("(c p) n -> p c n", p=P))
            wv_sb = wpool.tile([P, KC, D], bf16, tag="w", bufs=3, name="wv_sb")
            nc.sync.dma_start(out=wv_sb, in_=wv_d[:].rearrange("(c p) n -> p c n", p=P))
            for t in range(TCH):
                for n2 in range(2):
                    ps = ps_a.tile([P, 384], f32, tag="mm", bufs=4, name=f"psv{t}_{n2}")
                    for kc in range(KC):
                        nc.tensor.matmul(
                            ps,
                            lhsT=hT[:, kc, t * P : (t + 1) * P],
                            rhs=wv_sb[:, kc, n2 * 384 : (n2 + 1) * 384],
                            start=(kc == 0), stop=(kc == KC - 1),
                        )
                    nc.vector.tensor_copy(
                        out=vA[:, t, n2 * 6 : (n2 + 1) * 6, 0:HS],
                        in_=ps.rearrange("p (h d) -> p h d", h=6),
                    )
        ps_a.release()

        # ================= Phase 3: fused Q^T/K^T + attention =================
        # scores^T[k,q] per head pair (row-group packed, contraction=64),
        # fused scale+mask+exp into expT (bf16), PV with [V|1] -> O^T + denom.
        ps_b = tc.alloc_tile_pool(name="ps_b", bufs=1, space="PSUM")
        dpool = tc.alloc_tile_pool(name="dpool", bufs=1, space="DRAM")

        def emit_norm(pv, hp, qc):
            # deferred softmax-normalize: 1/denom (fast recip via SBUF copy),
            # replicate across partitions through a DRAM-roundtrip broadcast
            # DMA, then scale O rows during the PSUM evacuation.
            qs = slice(qc * 512, (qc + 1) * 512)
            pvr = rot.tile([1, 2, 512], f32, tag="pvr", bufs=2, name=f"pvr{hp}_{qc}")
            nc.vector.tensor_copy(out=pvr, in_=pv[HS : HS + 1, :, :])
            rsb = rot.tile([1, 2, 512], f32, tag="rsb", bufs=2, name=f"rsb{hp}_{qc}")
            for h in range(2):
                nc.vector.reciprocal_approx_fast(out=rsb[:, h, :], in_=pvr[:, h, :])
            rd = dpool.tile([1, 2, 512], f32, tag="rd", bufs=2, name=f"rd{hp}_{qc}")
            nc.sync.dma_start(out=rd, in_=rsb)
            rrs = rot.tile([HS, 2, 512], f32, tag="rrs", bufs=2, name=f"rrs{hp}_{qc}")
            nc.gpsimd.dma_start(
                out=rrs,
                in_=bass.AP(
                    tensor=rd.tensor, offset=rd.offset,
                    ap=[[0, HS]] + [list(a) for a in rd.ap[1:]],
                ),
            )
            nc.vector.tensor_tensor(
                oT[0:HS, hp, qs], pv[0:HS, 0, :], rrs[:, 0, :], OP.mult
            )
            nc.vector.tensor_tensor(
                oT[HS:P, hp, qs], pv[0:HS, 1, :], rrs[:, 1, :], OP.mult
            )

        with nc.named_scope("attn"):
            pending = None
            for hp in range(H // 2):
                # Q^T / K^T projection for this head pair (chunk hp) — emitted
                # inside the loop so its PE work overlaps attention's ACT-bound
                # stretches
                for n in range(NQ // 512):
                    psq = ps_b.tile([P, 2, 512], f32, tag="sc", bufs=2, name=f"psq{hp}_{n}")
                    for kc in range(KC):
                        nc.tensor.matmul(
                            psq[:, 0, :],
                            lhsT=wq_sb[:, kc, hp * P : (hp + 1) * P],
                            rhs=hT[:, kc, n * 512 : (n + 1) * 512],
                            start=(kc == 0), stop=(kc == KC - 1),
                        )
                    nc.vector.tensor_scalar_add(
                        out=qT[:, hp, n * 512 : (n + 1) * 512], in0=psq[:, 0, :],
                        scalar1=bq_sb[:, hp : hp + 1],
                    )
                for n in range(NT // 512):
                    psk = ps_b.tile([P, 2, 512], f32, tag="sc", bufs=2, name=f"psk{hp}_{n}")
                    for kc in range(KC):
                        nc.tensor.matmul(
                            psk[:, 0, :],
                            lhsT=wk_sb[:, kc, hp * P : (hp + 1) * P],
                            rhs=hT[:, kc, n * 512 : (n + 1) * 512],
                            start=(kc == 0), stop=(kc == KC - 1),
                        )
                    nc.vector.tensor_scalar_add(
                        out=kT[:, hp, n * 512 : (n + 1) * 512], in0=psk[:, 0, :],
                        scalar1=bk_sb[:, hp : hp + 1],
                    )
                for qc in range(NQ // 512):
                    qs = slice(qc * 512, (qc + 1) * 512)
                    pv = ps_b.tile([HS + 1, 2, 512], f32, tag="pv", bufs=2, name=f"pv{hp}_{qc}")
                    for j in range(TCH):
                        sc = ps_b.tile([P, 2, 512], f32, tag="sc", bufs=2, name=f"sc{hp}_{qc}_{j}")
                        js = slice(j * P, (j + 1) * P)
                        nc.tensor.matmul(
                            sc[:, 0, :], lhsT=kT[0:HS, hp, js], rhs=qT[0:HS, hp, qs],
                            start=True, stop=True,
                        )
                        nc.tensor.matmul(
                            sc[:, 1, :], lhsT=kT[HS:P, hp, js], rhs=qT[HS:P, hp, qs],
                            start=True, stop=True,
                        )
                        ex = rot.tile([P, 2, 512], bf16, tag="expT", bufs=3, name=f"ex{hp}_{qc}_{j}")
                        nc.scalar.activation(
                            ex, sc, AF.Exp, bias=mb_sb[:, j : j + 1], scale=SCALE,
                        )
                        for h in range(2):
                            nc.tensor.matmul(
                                pv[:, h, :],
                                lhsT=vA[:, j, 2 * hp + h, :],
                                rhs=ex[:, h, :],
                                start=(j == 0), stop=(j == TCH - 1),
                            )
                    if pending is not None:
                        emit_norm(*pending)
                    pending = (pv, hp, qc)
            emit_norm(*pending)
        hpool.release()
        apool.release()
        ps_b.release()
        dpool.release()

        # ================= Phase 4: out-projection + residual =================
        lpool = tc.alloc_tile_pool(name="lpool", bufs=1)
        ps_c = tc.alloc_tile_pool(name="ps_c", bufs=1, space="PSUM")
        x2 = lpool.tile([P, QCH, D], f32)
        h2T = lpool.tile([P, KC, NQ], bf16)
        gT = lpool.tile([P, KC, NQ], bf16)
        with nc.named_scope("proj"):
            wo_sb = wpool.tile([P, KC, D], bf16, tag="w", bufs=3, name="wo_sb")
            nc.sync.dma_start(out=wo_sb, in_=wo_d[:].rearrange("(c p) n -> p c n", p=P))
            for qm in range(QCH):
                for n2 in range(2):
                    ns = slice(n2 * 384, (n2 + 1) * 384)
                    ps = ps_c.tile([P, 384], f32, tag="mm", bufs=4, name=f"pso{qm}_{n2}")
                    for kc in range(KC):
                        nc.tensor.matmul(
                            ps,
                            lhsT=oT[:, kc, qm * P : (qm + 1) * P],
                            rhs=wo_sb[:, kc, ns],
                            start=(kc == 0), stop=(kc == KC - 1),
                        )
                    nc.vector.tensor_tensor(x2[:, qm, ns], ps, xq[:, qm, ns], OP.add)

        # ================= Phase 5: LN2 + transpose =================
        with nc.named_scope("ln2"):
            for t in range(QCH):
                scr = rot.tile([P, D], bf16, tag="xn", bufs=4, name=f"scr2_{t}")
                ssq = rot.tile([P, 1], f32, tag="ssq", bufs=4, name=f"ssq2_{t}")
                nc.scalar.activation(scr, x2[:, t], AF.Square, accum_out=ssq)
                msum = rot.tile([P, 1], f32, tag="msum", bufs=4, name=f"msum2_{t}")
                nc.vector.reduce_sum(out=msum, in_=x2[:, t], axis=mybir.AxisListType.X)
                mu = rot.tile([P, 1], f32, tag="mu", bufs=4, name=f"mu_2{t}")
                nc.vector.tensor_scalar_mul(out=mu, in0=msum, scalar1=1.0 / D)
                mu2 = rot.tile([P, 1], f32, tag="mu2", bufs=4, name=f"mu2_2{t}")
                nc.vector.tensor_tensor(mu2, mu, mu, OP.mult)
                ve = rot.tile([P, 1], f32, tag="ve", bufs=4, name=f"ve2_{t}")
                nc.vector.tensor_scalar(
                    out=ve, in0=ssq, scalar1=1.0 / D, scalar2=EPS,
                    op0=OP.mult, op1=OP.add,
                )
                nc.vector.tensor_tensor(ve, ve, mu2, OP.subtract)
                rstd = rot.tile([P, 1], f32, tag="rstd", bufs=4, name=f"rstd2_{t}")
                nc.vector.reciprocal_approx_fast(out=rstd, in_=ve)
                nc.scalar.activation(rstd, rstd, AF.Sqrt, scale=1.0)
                nmr = rot.tile([P, 1], f32, tag="nmr", bufs=4, name=f"nmr2_{t}")
                nc.vector.tensor_tensor(nmr, mu, rstd, OP.mult)
                nc.vector.tensor_scalar_mul(out=nmr, in0=nmr, scalar1=-1.0)
                xn = rot.tile([P, D], bf16, tag="xn", bufs=4, name=f"xn2_{t}")
                nc.scalar.activation(xn, x2[:, t], AF.Identity, bias=nmr, scale=rstd)
                pt = ps_c.tile([P, KC, P], bf16, tag="tp", bufs=3, name=f"pt2_{t}")
                for f in range(KC):
                    nc.tensor.transpose(pt[:, f], xn[:, f * P : (f + 1) * P], ident)
                nc.vector.tensor_copy(out=h2T[:, :, t * P : (t + 1) * P], in_=pt)
                # after LN2 consumed x2[t], fold the final-residual b2 in-place
                nc.vector.tensor_tensor(x2[:, t], x2[:, t], b2_b, OP.add)

        # ================= Phase 6: FFN =================
        with nc.named_scope("ffn"):
            w1_sb = wpool.tile([P, KC, D], bf16, tag="w", bufs=3, name="w1_sb")
            nc.sync.dma_start(out=w1_sb, in_=w1_d[:].rearrange("(c p) n -> p c n", p=P))
            for m in range(KC):
                for n in range(NQ // 512):
                    ps = ps_c.tile([P, 512], f32, tag="mm", bufs=4, name=f"psf{m}_{n}")
                    for kc in range(KC):
                        nc.tensor.matmul(
                            ps,
                            lhsT=w1_sb[:, kc, m * P : (m + 1) * P],
                            rhs=h2T[:, kc, n * 512 : (n + 1) * 512],
                            start=(kc == 0), stop=(kc == KC - 1),
                        )
                    nc.scalar.activation(
                        gT[:, m, n * 512 : (n + 1) * 512], ps, AF.Gelu,
                        bias=b1_sb[:, m : m + 1], scale=1.0,
                    )
            w2_sb = wpool.tile([P, KC, D], bf16, tag="w", bufs=3, name="w2_sb")
            nc.sync.dma_start(out=w2_sb, in_=w2_d[:].rearrange("(c p) n -> p c n", p=P))
            out_r = out_d[:].rearrange("(c p) d -> c p d", p=P)
            for qm in range(QCH):
                osb = rot.tile([P, D], f32, tag="osb", bufs=2, name=f"osb{qm}")
                for n2 in range(2):
                    ns = slice(n2 * 384, (n2 + 1) * 384)
                    ps = ps_c.tile([P, 384], f32, tag="mm", bufs=4, name=f"psg{qm}_{n2}")
                    for kc in range(KC):
                        nc.tensor.matmul(
                            ps,
                            lhsT=gT[:, kc, qm * P : (qm + 1) * P],
                            rhs=w2_sb[:, kc, ns],
                            start=(kc == 0), stop=(kc == KC - 1),
                        )
                    nc.vector.tensor_tensor(osb[:, ns], ps, x2[:, qm, ns], OP.add)
                nc.sync.dma_start(out=out_r[qm], in_=osb)
        lpool.release()
        ps_c.release()

    nc.finalize()
    return nc


def _prepare_in_maps(inputs):
    x = np.ascontiguousarray(np.asarray(inputs["x"], dtype=np.float32))
    mask = np.asarray(inputs["attention_mask"])
    ln1_g = np.asarray(inputs["ln1_g"], dtype=np.float64)
    ln1_b = np.asarray(inputs["ln1_b"], dtype=np.float64)
    ln2_g = np.asarray(inputs["ln2_g"], dtype=np.float64)
    ln2_b = np.asarray(inputs["ln2_b"], dtype=np.float64)
    Wq = np.asarray(inputs["Wq"], dtype=np.float64)
    Wk = np.asarray(inputs["Wk"], dtype=np.float64)
    Wv = np.asarray(inputs["Wv"], dtype=np.float64)
    Wo = np.asarray(inputs["Wo"], dtype=np.float64)
    W1 = np.asarray(inputs["W1"], dtype=np.float64)
    W2 = np.asarray(inputs["W2"], dtype=np.float64)
    bo = np.asarray(inputs["bo"], dtype=np.float64)
    b1 = np.asarray(inputs["b1"], dtype=np.float64)
    b2 = np.asarray(inputs["b2"], dtype=np.float64)

    # fold LN gains/biases into the projection weights
    wq_f = (ln1_g[:, None] * Wq).astype(BF16)
    wk_f = (ln1_g[:, None] * Wk).astype(BF16)
    wv_f = (ln1_g[:, None] * Wv).astype(BF16)
    bq = (ln1_b @ Wq).astype(np.float32)
    bk = (ln1_b @ Wk).astype(np.float32)
    bv = ln1_b @ Wv
    wo_f = Wo.astype(BF16)
    bo2 = (bo + bv @ Wo).astype(np.float32)  # V-bias adds uniformly post-softmax
    w1_f = (ln2_g[:, None] * W1).astype(BF16)
    b1f = (b1 + ln2_b @ W1).astype(np.float32)
    w2_f = W2.astype(BF16)
    b2f = b2.astype(np.float32)

    maskbias = np.where(mask == 0, np.float32(NEG), np.float32(0.0)).astype(np.float32)

    in_maps = []
    for c in range(8):
        b, half = divmod(c, 2)
        xb = np.roll(x[b], -half * NQ, axis=0)
        mbb = np.roll(maskbias[b], -half * NQ, axis=0)
        in_maps.append(
            {
                "x_local": np.ascontiguousarray(xb),
                "maskbias": np.ascontiguousarray(mbb),
                "wq": wq_f, "wk": wk_f, "wv": wv_f, "wo": wo_f,
                "w1": w1_f, "w2": w2_f,
                "bq": bq, "bk": bk, "bo2": bo2, "b1f": b1f, "b2f": b2f,
            }
        )
    return in_maps


def run_on_cores(inputs, **spmd_kwargs):
    """Build (cached), run on cores 0-7, return (full_output, BassKernelResults)."""
    from concourse.bass_utils import run_bass_kernel_spmd

    if "nc" not in _PROGRAM_CACHE:
        _PROGRAM_CACHE["nc"] = _build_program()
    nc = _PROGRAM_CACHE["nc"]
    in_maps = _prepare_in_maps(inputs)
    res = run_bass_kernel_spmd(nc, in_maps, core_ids=list(range(8)), **spmd_kwargs)
    out = np.empty((B, S, D), dtype=np.float32)
    for c in range(8):
        b, half = divmod(c, 2)
        out[b, half * NQ : (half + 1) * NQ] = res.results[c]["out"]
    return out, res


def kernel(**inputs):
    out, _ = run_on_cores(inputs)
    return out
